# revision 20
# baseline (speedup 1.0000x reference)
"""Belief-propagation single-iteration kernel for 8 Trainium2 NeuronCores.

Problem (see reference):
    theta0: (2048, 8192) f32   clique A over (v0, v1_fine)
    theta1: (4096, 2048) f32   clique B over (v1_coarse, v2)
    idx_a, idx_b: (8192,) int64 maps fine->coarse

    marg_a = logsumexp(theta0, axis=0)                      # (8192,)
    msg_ab = segment_logsumexp(marg_a, idx_b, 4096)         # (4096,)
    marg_b = logsumexp(theta1, axis=1)                      # (4096,)
    msg_ba = marg_b[idx_a]                                  # (8192,)
    t0 = theta0 + msg_ba[None, :];  t0 -= logsumexp(t0)
    t1 = theta1 + msg_ab[:, None];  t1 -= logsumexp(t1)
    return (t0, t1)

Device algorithm works in the linear "sum-exp" domain:
    S_a[j] = sum_i exp(theta0[i,j])        (AllReduce over row shards)
    S_b[c] = sum_k exp(theta1[c,k])        (AllReduce over col shards)
    seg[c] = sum_{j: idx_b[j]=c} S_a[j]    (CSR-padded ap_gather + reduce)
    dot0 = sum_j S_a[j] * S_b[idx_a[j]]  = exp(Z0)
    dot1 = sum_c S_b[c] * seg[c]         = exp(Z1)
    t0 = theta0 + ln(S_b[idx_a[j]] / dot0)   (broadcast add over rows)
    t1 = theta1 + ln(seg[c] / dot1)          (broadcast add over cols)

Sharding: theta0 row-sharded (v0), theta1 column-sharded (v2); the small
marginal vectors are AllReduced and the message computation runs
replicated on every core.  Fully SPMD - no core-id dependence anywhere.

The S_b table is stored in a "sigma" permuted order sigma(c) =
(c % 128) * 32 + c // 128 so that the per-core [128, 32] partial-sum
tile DMAs to the collective buffer with a plain access pattern; idx_a
is remapped on the host to compensate.

Partition-broadcasts of the small tables use K=1 ones-matmuls on the PE
(a 0-stride broadcast DMA measures ~65 GB/s - far too slow).  The
msg_ba gather is split 8 ways across the GPSIMD cores (ap_gather costs
~20 ns per index per core), then reassembled through a DRAM row and
rebroadcast by matmul, with ln() applied on the PSUM chunks.
"""

import numpy as np

D0, D1F, D1C, D2 = 2048, 8192, 4096, 2048
NCORES = 8
R0 = D0 // NCORES  # theta0 rows per core (256)
C2 = D2 // NCORES  # theta1 cols per core (256)
T0T = R0 // 128  # theta0 SBUF tiles per core (2)
T1T = D1C // 128  # theta1 row tiles (32)
NCHUNK = D1F // 512  # 512-wide chunks of the fine axis (16)
PAD = 128  # zero slots appended to the S_a gather table

_cache = {}


def _build(cap, repeats=1, stage=99):
    import concourse.bacc as bacc
    import concourse.tile as tile
    from concourse import mybir

    f32 = mybir.dt.float32
    i16 = mybir.dt.int16
    Exp = mybir.ActivationFunctionType.Exp
    Ln = mybir.ActivationFunctionType.Ln
    X = mybir.AxisListType.X
    add = mybir.AluOpType.add

    nc = bacc.Bacc("TRN2", target_bir_lowering=False, debug=False, num_devices=NCORES)

    th0 = nc.dram_tensor("th0", [R0, D1F], f32, kind="ExternalInput").ap()
    th1 = nc.dram_tensor("th1", [D1C, C2], f32, kind="ExternalInput").ap()
    idxa = nc.dram_tensor("idxa", [128, D1F // 128], i16, kind="ExternalInput").ap()
    lob = nc.dram_tensor("lob", [128, 64], f32, kind="ExternalInput").ap()
    hib = nc.dram_tensor("hib", [128, 64], f32, kind="ExternalInput").ap()
    iota = nc.dram_tensor("iota", [128, 128], f32, kind="ExternalInput").ap()
    t0 = nc.dram_tensor("t0", [R0, D1F], f32, kind="ExternalOutput").ap()
    t1 = nc.dram_tensor("t1", [D1C, C2], f32, kind="ExternalOutput").ap()

    with tile.TileContext(nc) as tc:
        with (
            tc.tile_pool(name="singles", bufs=1) as singles,
            tc.tile_pool(name="big", bufs=1) as big,
            tc.tile_pool(name="stg", bufs=3) as stg,
            tc.tile_pool(name="lnp", bufs=3) as lnp,
            tc.tile_pool(name="psum", bufs=4, space="PSUM") as psum,
            tc.tile_pool(name="pss", bufs=1, space="PSUM") as pss,
            tc.tile_pool(name="dram", bufs=1, space="DRAM") as dram,
        ):
            ones = singles.tile([128, 128], f32)
            nc.vector.memset(ones, 1.0)
            ones_col = singles.tile([1, 128], f32)
            nc.vector.memset(ones_col, 1.0)
            idxa_sb = singles.tile([128, D1F // 128], i16)
            nc.sync.dma_start(out=idxa_sb, in_=idxa)
            lob_sb = singles.tile([128, 64], f32)
            nc.sync.dma_start(out=lob_sb, in_=lob)
            hib_sb = singles.tile([128, 64], f32)
            nc.sync.dma_start(out=hib_sb, in_=hib)
            iota_sb = singles.tile([128, 128], f32)
            nc.sync.dma_start(out=iota_sb, in_=iota)

            for _rep in range(repeats):
                th0_sb = big.tile([128, T0T, D1F], f32, tag="th0")
                th1_sb = big.tile([128, T1T, C2], f32, tag="th1")
                sb_tbl = big.tile([128, D1C], f32, tag="sbt")
                row_sb = singles.tile([1, D1F], f32, tag="row")
                mb_g = singles.tile([128, D1F // 8], f32, tag="mbg")
                sb_col = singles.tile([128, T1T], f32, tag="sbcol")
                sbl_col = singles.tile([128, T1T], f32, tag="sblcol")
                prod1 = singles.tile([128, T1T], f32, tag="prod1")
                sa64 = singles.tile([128, 64], f32, tag="sa64")
                mb64 = singles.tile([128, 64], f32, tag="mb64")
                d0p = singles.tile([128, 1], f32, tag="d0p")
                recip0 = singles.tile([128, 1], f32, tag="rec0")
                recip1 = singles.tile([128, 1], f32, tag="rec1")
                pr_col = singles.tile([128, 1], f32, tag="prc")
                mg_col = singles.tile([128, T1T], f32, tag="mgc")

                sa_jt = singles.tile([128, 64], f32, tag="sajt")

                ccb_in = dram.tile([1, D1C], f32, tag="ccbin")
                ccb_out = dram.tile([1, D1C], f32, tag="ccbout")
                cca_in = dram.tile([1, D1F], f32, tag="ccain")
                cca_out = dram.tile([1, D1F], f32, tag="ccaout")
                mrow_d = dram.tile([1, D1F], f32, tag="mrow")

                # ---- phase A: theta1 load -> S_b partials ----------------
                for t in range(T1T):
                    nc.sync.dma_start(out=th1_sb[:, t, :],
                                      in_=th1[128 * t:128 * (t + 1), :])
                    e1 = stg.tile([128, C2], f32, tag="t1e")
                    nc.scalar.activation(e1, th1_sb[:, t, :], Exp)
                    nc.vector.tensor_reduce(sb_col[:, t:t + 1], e1, axis=X, op=add)
                # natural [p, t] layout == sigma order sigma(c)=(c%128)*32+c//128
                nc.sync.dma_start(out=ccb_in[0, :], in_=sb_col)
                nc.gpsimd.collective_compute(
                    "AllReduce",
                    mybir.AluOpType.add,
                    replica_groups=[list(range(NCORES))],
                    ins=[ccb_in.opt()],
                    outs=[ccb_out.opt()],
                )

                # ---- S_b broadcast + msg_ba gather (overlaps phase B) ----
                nc.sync.dma_start(out=row_sb[0:1, :D1C], in_=ccb_out[0:1, :])
                for n in range(D1C // 512):
                    sl = slice(512 * n, 512 * (n + 1))
                    ps = psum.tile([128, 512], f32, tag="ps")
                    nc.tensor.matmul(ps, ones_col, row_sb[0:1, sl],
                                     start=True, stop=True)
                    if n % 2 == 0:
                        nc.vector.tensor_copy(sb_tbl[:, sl], ps)
                    else:
                        nc.scalar.copy(sb_tbl[:, sl], ps)
                if stage >= 5:
                    # msg_ba gather, split 8 ways: core k's partition group
                    # gathers S_b[sigma(idx_a[j])] for j in [1024k, 1024k+1024)
                    nc.gpsimd.ap_gather(
                        out_ap=mb_g[:].rearrange("p (n d) -> p n d", d=1),
                        in_ap=sb_tbl[:].rearrange("p (n d) -> p n d", d=1),
                        idxs_ap=idxa_sb[:],
                        channels=128, num_elems=D1C, d=1, num_idxs=D1F // 8)
                    for k in range(8):
                        nc.sync.dma_start(
                            out=mrow_d[0, 1024 * k:1024 * (k + 1)],
                            in_=mb_g[16 * k:16 * k + 1, :])
                    nc.sync.dma_start(
                        out=mb64,
                        in_=mrow_d[0, :].rearrange("(p t) -> p t", t=64))
                nc.sync.dma_start(
                    out=sbl_col,
                    in_=ccb_out[0, :].rearrange("(p t) -> p t", t=T1T))

                if stage < 2:
                    continue
                # ---- phase B: theta0 stream in -> S_a partials -----------
                for n in range(NCHUNK):
                    sl = slice(512 * n, 512 * (n + 1))
                    for t in range(T0T):
                        nc.sync.dma_start(
                            out=th0_sb[:, t, sl],
                            in_=th0[128 * t:128 * (t + 1), sl])
                    e0 = stg.tile([128, 512], f32, tag="e0")
                    e1b = stg.tile([128, 512], f32, tag="e1b")
                    nc.scalar.activation(e0, th0_sb[:, 0, sl], Exp)
                    nc.scalar.activation(e1b, th0_sb[:, 1, sl], Exp)
                    ps = psum.tile([128, 512], f32, tag="ps")
                    nc.tensor.matmul(ps, ones, e0, start=True, stop=False)
                    nc.tensor.matmul(ps, ones, e1b, start=False, stop=True)
                    # every PSUM row holds the same column sums; ship row 0
                    sa_row = stg.tile([1, 512], f32, tag="sarow")
                    nc.scalar.copy(sa_row, ps[0:1, :])
                    nc.sync.dma_start(out=cca_in[0, sl], in_=sa_row)

                if stage < 3:
                    continue
                # ---- AllReduce (S_a) -------------------------------------
                nc.gpsimd.collective_compute(
                    "AllReduce",
                    mybir.AluOpType.add,
                    replica_groups=[list(range(NCORES))],
                    ins=[cca_in.opt()],
                    outs=[cca_out.opt()],
                )

                if stage < 4:
                    continue
                # small c-layout vectors for the normalizer dots
                nc.sync.dma_start(
                    out=sa64,
                    in_=cca_out[0, :].rearrange("(p t) -> p t", t=64))
                # S_a in (j % 128, j // 128) layout for the one-hot seg sum
                nc.sync.dma_start(
                    out=sa_jt,
                    in_=cca_out[0, :].rearrange("(t p) -> p t", p=128))
                if stage >= 5:
                    # segment sums on the PE: psum_seg[p, t] = seg[128t + p]
                    #   = sum_j 1[idx_b[j] % 128 = p] * 1[idx_b[j]//128 = t]
                    #         * S_a[j]
                    psum_seg = pss.tile([128, T1T], f32, tag="segps")
                    for jt in range(64):
                        loh = stg.tile([128, 128], f32, tag="loh")
                        nc.vector.tensor_scalar(
                            loh, iota_sb, lob_sb[:, jt:jt + 1], None,
                            op0=mybir.AluOpType.is_equal)
                        him = stg.tile([128, T1T], f32, tag="him")
                        nc.vector.tensor_scalar(
                            him, iota_sb[:, :T1T], hib_sb[:, jt:jt + 1],
                            sa_jt[:, jt:jt + 1],
                            op0=mybir.AluOpType.is_equal,
                            op1=mybir.AluOpType.mult)
                        nc.tensor.matmul(psum_seg, loh, him,
                                         start=(jt == 0), stop=(jt == 63))

                if stage < 6:
                    continue
                # ---- t0 path: dot0, rebroadcast msg_ba row, ln, add ------
                nc.vector.tensor_mul(mb64, mb64, sa64)
                nc.vector.tensor_reduce(d0p, mb64, axis=X, op=add)
                d0ps = pss.tile([128, 1], f32, tag="d0ps")
                nc.tensor.matmul(d0ps, ones, d0p, start=True, stop=True)
                nc.vector.reciprocal(out=recip0, in_=d0ps)

                nc.sync.dma_start(out=row_sb[0:1, :], in_=mrow_d[0:1, :])
                for n in range(NCHUNK):
                    sl = slice(512 * n, 512 * (n + 1))
                    ps = psum.tile([128, 512], f32, tag="ps")
                    nc.tensor.matmul(ps, ones_col, row_sb[0:1, sl],
                                     start=True, stop=True)
                    lnc = lnp.tile([128, 512], f32, tag="lnc")
                    # msgba_adj = ln(S_b[idx_a[j]] / dot0), from PSUM
                    nc.scalar.activation(lnc, ps, Ln, scale=recip0[:, 0:1])
                    for t in range(T0T):
                        nc.vector.tensor_add(th0_sb[:, t, sl],
                                             th0_sb[:, t, sl], lnc)
                        nc.sync.dma_start(out=t0[128 * t:128 * (t + 1), sl],
                                          in_=th0_sb[:, t, sl])

                if stage < 7:
                    continue
                # ---- t1 path: dot1, ln, add ------------------------------
                nc.vector.tensor_mul(prod1, psum_seg, sbl_col)
                nc.vector.tensor_reduce(pr_col, prod1, axis=X, op=add)
                prps = pss.tile([128, 1], f32, tag="prps")
                nc.tensor.matmul(prps, ones, pr_col, start=True, stop=True)
                nc.vector.reciprocal(out=recip1, in_=prps)
                # msgab_col[p, t] = ln(seg[128t+p] / dot1)
                nc.scalar.activation(mg_col, psum_seg, Ln, scale=recip1[:, 0:1])
                for t in range(T1T):
                    nc.vector.tensor_scalar_add(th1_sb[:, t, :], th1_sb[:, t, :],
                                                mg_col[:, t:t + 1])
                    nc.sync.dma_start(out=t1[128 * t:128 * (t + 1), :],
                                      in_=th1_sb[:, t, :])

    nc.compile()
    return nc


def _prep_inputs(theta0, theta1, idx_a, idx_b):
    """Host-side sharding + index-table construction (no float math)."""
    theta0 = np.ascontiguousarray(np.asarray(theta0, dtype=np.float32))
    theta1 = np.ascontiguousarray(np.asarray(theta1, dtype=np.float32))
    ia = np.asarray(idx_a).astype(np.int64)
    ib = np.asarray(idx_b).astype(np.int64)

    # sigma-permuted idx_a (S_b table is stored in sigma order), split 8 ways:
    # core k's group holds j in [1024k, 1024k+1024) wrapped over 16 partitions
    sig_ia = ((ia % 128) * 32 + ia // 128).astype(np.int16)
    idxa_w = np.ascontiguousarray(
        sig_ia.reshape(8, 64, 16).transpose(0, 2, 1).reshape(128, 64))

    # idx_b lo/hi tables in (j % 128, j // 128) layout for the one-hot
    # segment sum: seg lands as [p = c % 128, t = c // 128]
    ibm = ib.reshape(64, 128).T
    lob_w = np.ascontiguousarray((ibm % 128).astype(np.float32))
    hib_w = np.ascontiguousarray((ibm // 128).astype(np.float32))
    iota_w = np.tile(np.arange(128, dtype=np.float32)[None, :], (128, 1))

    in_maps = []
    for k in range(NCORES):
        in_maps.append({
            "th0": theta0[R0 * k:R0 * (k + 1), :],
            "th1": np.ascontiguousarray(theta1[:, C2 * k:C2 * (k + 1)]),
            "idxa": idxa_w,
            "lob": lob_w,
            "hib": hib_w,
            "iota": iota_w,
        })
    return 0, in_maps


def get_program(cap, repeats=1):
    key = ("prog", cap, repeats)
    if key not in _cache:
        _cache[key] = _build(cap, repeats)
    return _cache[key]


def kernel(theta0, theta1, idx_a, idx_b):
    from concourse.bass_utils import run_bass_kernel_spmd

    cap, in_maps = _prep_inputs(theta0, theta1, idx_a, idx_b)
    nc = get_program(cap)
    res = run_bass_kernel_spmd(nc, in_maps, list(range(NCORES)))
    t0 = np.concatenate([res.results[k]["t0"] for k in range(NCORES)], axis=0)
    t1 = np.concatenate([res.results[k]["t1"] for k in range(NCORES)], axis=1)
    return (t0, t1)


# revision 21
# speedup vs baseline: 2.1513x; 2.1513x over previous
"""Belief-propagation single-iteration kernel for 8 Trainium2 NeuronCores.

Problem (see reference):
    theta0: (2048, 8192) f32   clique A over (v0, v1_fine)
    theta1: (4096, 2048) f32   clique B over (v1_coarse, v2)
    idx_a, idx_b: (8192,) int64 maps fine->coarse

    marg_a = logsumexp(theta0, axis=0)                      # (8192,)
    msg_ab = segment_logsumexp(marg_a, idx_b, 4096)         # (4096,)
    marg_b = logsumexp(theta1, axis=1)                      # (4096,)
    msg_ba = marg_b[idx_a]                                  # (8192,)
    t0 = theta0 + msg_ba[None, :];  t0 -= logsumexp(t0)
    t1 = theta1 + msg_ab[:, None];  t1 -= logsumexp(t1)
    return (t0, t1)

Device algorithm works in the linear "sum-exp" domain:
    S_a[j] = sum_i exp(theta0[i,j])        (AllReduce over row shards)
    S_b[c] = sum_k exp(theta1[c,k])        (AllReduce over col shards)
    seg[c] = sum_{j: idx_b[j]=c} S_a[j]    (CSR-padded ap_gather + reduce)
    dot0 = sum_j S_a[j] * S_b[idx_a[j]]  = exp(Z0)
    dot1 = sum_c S_b[c] * seg[c]         = exp(Z1)
    t0 = theta0 + ln(S_b[idx_a[j]] / dot0)   (broadcast add over rows)
    t1 = theta1 + ln(seg[c] / dot1)          (broadcast add over cols)

Sharding: theta0 row-sharded (v0), theta1 column-sharded (v2); the small
marginal vectors are AllReduced and the message computation runs
replicated on every core.  Fully SPMD - no core-id dependence anywhere.

The S_b table is stored in a "sigma" permuted order sigma(c) =
(c % 128) * 32 + c // 128 so that the per-core [128, 32] partial-sum
tile DMAs to the collective buffer with a plain access pattern; idx_a
is remapped on the host to compensate.

Partition-broadcasts of the small tables use K=1 ones-matmuls on the PE
(a 0-stride broadcast DMA measures ~65 GB/s - far too slow).  The
msg_ba gather is split 8 ways across the GPSIMD cores (ap_gather costs
~20 ns per index per core), then reassembled through a DRAM row and
rebroadcast by matmul, with ln() applied on the PSUM chunks.
"""

import numpy as np

D0, D1F, D1C, D2 = 2048, 8192, 4096, 2048
NCORES = 8
R0 = D0 // NCORES  # theta0 rows per core (256)
C2 = D2 // NCORES  # theta1 cols per core (256)
T0T = R0 // 128  # theta0 SBUF tiles per core (2)
T1T = D1C // 128  # theta1 row tiles (32)
NCHUNK = D1F // 512  # 512-wide chunks of the fine axis (16)
PAD = 128  # zero slots appended to the S_a gather table

_cache = {}


def _build(cap, repeats=1, stage=99):
    import concourse.bacc as bacc
    import concourse.tile as tile
    from concourse import mybir

    f32 = mybir.dt.float32
    i16 = mybir.dt.int16
    Exp = mybir.ActivationFunctionType.Exp
    Ln = mybir.ActivationFunctionType.Ln
    X = mybir.AxisListType.X
    add = mybir.AluOpType.add

    nc = bacc.Bacc("TRN2", target_bir_lowering=False, debug=False, num_devices=NCORES)

    th0 = nc.dram_tensor("th0", [R0, D1F], f32, kind="ExternalInput").ap()
    th1 = nc.dram_tensor("th1", [D1C, C2], f32, kind="ExternalInput").ap()
    idxa = nc.dram_tensor("idxa", [128, D1F // 128], i16, kind="ExternalInput").ap()
    lob = nc.dram_tensor("lob", [128, 64], f32, kind="ExternalInput").ap()
    hib = nc.dram_tensor("hib", [128, 64], f32, kind="ExternalInput").ap()
    iota = nc.dram_tensor("iota", [128, 128], f32, kind="ExternalInput").ap()
    t0 = nc.dram_tensor("t0", [R0, D1F], f32, kind="ExternalOutput").ap()
    t1 = nc.dram_tensor("t1", [D1C, C2], f32, kind="ExternalOutput").ap()

    with tile.TileContext(nc) as tc:
        with (
            tc.tile_pool(name="singles", bufs=1) as singles,
            tc.tile_pool(name="big", bufs=1) as big,
            tc.tile_pool(name="stg", bufs=3) as stg,
            tc.tile_pool(name="lnp", bufs=3) as lnp,
            tc.tile_pool(name="psum", bufs=4, space="PSUM") as psum,
            tc.tile_pool(name="pss", bufs=1, space="PSUM") as pss,
            tc.tile_pool(name="dram", bufs=1, space="DRAM") as dram,
        ):
            ones = singles.tile([128, 128], f32)
            nc.vector.memset(ones, 1.0)
            ones_col = singles.tile([1, 128], f32)
            nc.vector.memset(ones_col, 1.0)
            idxa_sb = singles.tile([128, D1F // 128], i16)
            nc.sync.dma_start(out=idxa_sb, in_=idxa)
            lob_sb = singles.tile([128, 64], f32)
            nc.sync.dma_start(out=lob_sb, in_=lob)
            hib_sb = singles.tile([128, 64], f32)
            nc.sync.dma_start(out=hib_sb, in_=hib)
            iota_sb = singles.tile([128, 128], f32)
            nc.sync.dma_start(out=iota_sb, in_=iota)

            for _rep in range(repeats):
                th0_sb = big.tile([128, T0T, D1F], f32, tag="th0")
                th1_sb = big.tile([128, T1T, C2], f32, tag="th1")
                sb_tbl = big.tile([128, D1C], f32, tag="sbt")
                row_sb = singles.tile([1, D1F], f32, tag="row")
                mb_g = singles.tile([128, D1F // 8], f32, tag="mbg")
                sb_col = singles.tile([128, T1T], f32, tag="sbcol")
                sbl_col = singles.tile([128, T1T], f32, tag="sblcol")
                prod1 = singles.tile([128, T1T], f32, tag="prod1")
                sa64 = singles.tile([128, 64], f32, tag="sa64")
                mb64 = singles.tile([128, 64], f32, tag="mb64")
                d0p = singles.tile([128, 1], f32, tag="d0p")
                recip0 = singles.tile([128, 1], f32, tag="rec0")
                recip1 = singles.tile([128, 1], f32, tag="rec1")
                pr_col = singles.tile([128, 1], f32, tag="prc")
                mg_col = singles.tile([128, T1T], f32, tag="mgc")

                sa_jt = singles.tile([128, 64], f32, tag="sajt")

                ccb_in = dram.tile([1, D1C], f32, tag="ccbin")
                ccb_out = dram.tile([1, D1C], f32, tag="ccbout")
                cca_in = dram.tile([1, D1F], f32, tag="ccain")
                cca_out = dram.tile([1, D1F], f32, tag="ccaout")
                mrow_d = dram.tile([1, D1F], f32, tag="mrow")

                # ---- phase A: theta1 load -> S_b partials ----------------
                for t in range(T1T):
                    nc.sync.dma_start(out=th1_sb[:, t, :],
                                      in_=th1[128 * t:128 * (t + 1), :])
                    e1 = stg.tile([128, C2], f32, tag="t1e")
                    nc.scalar.activation(e1, th1_sb[:, t, :], Exp)
                    nc.vector.tensor_reduce(sb_col[:, t:t + 1], e1, axis=X, op=add)
                # natural [p, t] layout == sigma order sigma(c)=(c%128)*32+c//128
                nc.sync.dma_start(out=ccb_in[0, :], in_=sb_col)
                nc.gpsimd.collective_compute(
                    "AllReduce",
                    mybir.AluOpType.add,
                    replica_groups=[list(range(NCORES))],
                    ins=[ccb_in.opt()],
                    outs=[ccb_out.opt()],
                )

                if stage < 2:
                    continue
                # ---- phase B: theta0 stream in -> S_a partials -----------
                for n in range(NCHUNK):
                    sl = slice(512 * n, 512 * (n + 1))
                    for t in range(T0T):
                        nc.sync.dma_start(
                            out=th0_sb[:, t, sl],
                            in_=th0[128 * t:128 * (t + 1), sl])
                    e0 = stg.tile([128, 512], f32, tag="e0")
                    e1b = stg.tile([128, 512], f32, tag="e1b")
                    nc.scalar.activation(e0, th0_sb[:, 0, sl], Exp)
                    nc.scalar.activation(e1b, th0_sb[:, 1, sl], Exp)
                    nc.vector.tensor_add(e0, e0, e1b)
                    ps = psum.tile([128, 512], f32, tag="ps")
                    nc.tensor.matmul(ps, ones, e0, start=True, stop=True)
                    # every PSUM row holds the same column sums; ship row 0
                    sa_row = stg.tile([1, 512], f32, tag="sarow")
                    nc.vector.tensor_copy(sa_row, ps[0:1, :])
                    nc.sync.dma_start(out=cca_in[0, sl], in_=sa_row)

                if stage < 3:
                    continue
                # ---- AllReduce (S_a) -------------------------------------
                nc.gpsimd.collective_compute(
                    "AllReduce",
                    mybir.AluOpType.add,
                    replica_groups=[list(range(NCORES))],
                    ins=[cca_in.opt()],
                    outs=[cca_out.opt()],
                )

                if stage < 4:
                    continue
                # ---- S_b broadcast + msg_ba gather -----------------------
                nc.sync.dma_start(out=row_sb[0:1, :D1C], in_=ccb_out[0:1, :])
                for n in range(D1C // 512):
                    sl = slice(512 * n, 512 * (n + 1))
                    ps = psum.tile([128, 512], f32, tag="ps")
                    nc.tensor.matmul(ps, ones_col, row_sb[0:1, sl],
                                     start=True, stop=True)
                    if n % 2 == 0:
                        nc.vector.tensor_copy(sb_tbl[:, sl], ps)
                    else:
                        nc.scalar.copy(sb_tbl[:, sl], ps)
                if stage >= 5:
                    # msg_ba gather, split 8 ways: core k's partition group
                    # gathers S_b[sigma(idx_a[j])] for j in [1024k, 1024k+1024)
                    nc.gpsimd.ap_gather(
                        out_ap=mb_g[:].rearrange("p (n d) -> p n d", d=1),
                        in_ap=sb_tbl[:].rearrange("p (n d) -> p n d", d=1),
                        idxs_ap=idxa_sb[:],
                        channels=128, num_elems=D1C, d=1, num_idxs=D1F // 8)
                    for k in range(8):
                        nc.sync.dma_start(
                            out=mrow_d[0, 1024 * k:1024 * (k + 1)],
                            in_=mb_g[16 * k:16 * k + 1, :])
                    nc.sync.dma_start(
                        out=mb64,
                        in_=mrow_d[0, :].rearrange("(p t) -> p t", t=64))
                nc.sync.dma_start(
                    out=sbl_col,
                    in_=ccb_out[0, :].rearrange("(p t) -> p t", t=T1T))
                # small c-layout vectors for the normalizer dots
                nc.sync.dma_start(
                    out=sa64,
                    in_=cca_out[0, :].rearrange("(p t) -> p t", t=64))
                # S_a in (j % 128, j // 128) layout for the one-hot seg sum
                nc.sync.dma_start(
                    out=sa_jt,
                    in_=cca_out[0, :].rearrange("(t p) -> p t", p=128))
                if stage >= 5:
                    # segment sums on the PE: psum_seg[p, t] = seg[128t + p]
                    #   = sum_j 1[idx_b[j] % 128 = p] * 1[idx_b[j]//128 = t]
                    #         * S_a[j]
                    psum_seg = pss.tile([128, T1T], f32, tag="segps")
                    for jt in range(64):
                        loh = stg.tile([128, 128], f32, tag="loh")
                        nc.vector.tensor_scalar(
                            loh, iota_sb, lob_sb[:, jt:jt + 1], None,
                            op0=mybir.AluOpType.is_equal)
                        him = stg.tile([128, T1T], f32, tag="him")
                        nc.vector.tensor_scalar(
                            him, iota_sb[:, :T1T], hib_sb[:, jt:jt + 1],
                            sa_jt[:, jt:jt + 1],
                            op0=mybir.AluOpType.is_equal,
                            op1=mybir.AluOpType.mult)
                        nc.tensor.matmul(psum_seg, loh, him,
                                         start=(jt == 0), stop=(jt == 63))

                if stage < 6:
                    continue
                # ---- t0 path: dot0, rebroadcast msg_ba row, ln, add ------
                nc.vector.tensor_mul(mb64, mb64, sa64)
                nc.vector.tensor_reduce(d0p, mb64, axis=X, op=add)
                d0ps = pss.tile([128, 1], f32, tag="d0ps")
                nc.tensor.matmul(d0ps, ones, d0p, start=True, stop=True)
                nc.vector.reciprocal(out=recip0, in_=d0ps)

                nc.sync.dma_start(out=row_sb[0:1, :], in_=mrow_d[0:1, :])
                for n in range(NCHUNK):
                    sl = slice(512 * n, 512 * (n + 1))
                    ps = psum.tile([128, 512], f32, tag="ps")
                    nc.tensor.matmul(ps, ones_col, row_sb[0:1, sl],
                                     start=True, stop=True)
                    lnc = lnp.tile([128, 512], f32, tag="lnc")
                    # msgba_adj = ln(S_b[idx_a[j]] / dot0), from PSUM
                    nc.scalar.activation(lnc, ps, Ln, scale=recip0[:, 0:1])
                    for t in range(T0T):
                        nc.vector.tensor_add(th0_sb[:, t, sl],
                                             th0_sb[:, t, sl], lnc)
                        nc.sync.dma_start(out=t0[128 * t:128 * (t + 1), sl],
                                          in_=th0_sb[:, t, sl])

                if stage < 7:
                    continue
                # ---- t1 path: dot1, ln, add ------------------------------
                nc.vector.tensor_mul(prod1, psum_seg, sbl_col)
                nc.vector.tensor_reduce(pr_col, prod1, axis=X, op=add)
                prps = pss.tile([128, 1], f32, tag="prps")
                nc.tensor.matmul(prps, ones, pr_col, start=True, stop=True)
                nc.vector.reciprocal(out=recip1, in_=prps)
                # msgab_col[p, t] = ln(seg[128t+p] / dot1)
                nc.scalar.activation(mg_col, psum_seg, Ln, scale=recip1[:, 0:1])
                for t in range(T1T):
                    nc.vector.tensor_scalar_add(th1_sb[:, t, :], th1_sb[:, t, :],
                                                mg_col[:, t:t + 1])
                    nc.sync.dma_start(out=t1[128 * t:128 * (t + 1), :],
                                      in_=th1_sb[:, t, :])

    nc.compile()
    return nc


def _prep_inputs(theta0, theta1, idx_a, idx_b):
    """Host-side sharding + index-table construction (no float math)."""
    theta0 = np.ascontiguousarray(np.asarray(theta0, dtype=np.float32))
    theta1 = np.ascontiguousarray(np.asarray(theta1, dtype=np.float32))
    ia = np.asarray(idx_a).astype(np.int64)
    ib = np.asarray(idx_b).astype(np.int64)

    # sigma-permuted idx_a (S_b table is stored in sigma order), split 8 ways:
    # core k's group holds j in [1024k, 1024k+1024) wrapped over 16 partitions
    sig_ia = ((ia % 128) * 32 + ia // 128).astype(np.int16)
    idxa_w = np.ascontiguousarray(
        sig_ia.reshape(8, 64, 16).transpose(0, 2, 1).reshape(128, 64))

    # idx_b lo/hi tables in (j % 128, j // 128) layout for the one-hot
    # segment sum: seg lands as [p = c % 128, t = c // 128]
    ibm = ib.reshape(64, 128).T
    lob_w = np.ascontiguousarray((ibm % 128).astype(np.float32))
    hib_w = np.ascontiguousarray((ibm // 128).astype(np.float32))
    iota_w = np.tile(np.arange(128, dtype=np.float32)[None, :], (128, 1))

    in_maps = []
    for k in range(NCORES):
        in_maps.append({
            "th0": theta0[R0 * k:R0 * (k + 1), :],
            "th1": np.ascontiguousarray(theta1[:, C2 * k:C2 * (k + 1)]),
            "idxa": idxa_w,
            "lob": lob_w,
            "hib": hib_w,
            "iota": iota_w,
        })
    return 0, in_maps


def get_program(cap, repeats=1):
    key = ("prog", cap, repeats)
    if key not in _cache:
        _cache[key] = _build(cap, repeats)
    return _cache[key]


def kernel(theta0, theta1, idx_a, idx_b):
    from concourse.bass_utils import run_bass_kernel_spmd

    cap, in_maps = _prep_inputs(theta0, theta1, idx_a, idx_b)
    nc = get_program(cap)
    res = run_bass_kernel_spmd(nc, in_maps, list(range(NCORES)))
    t0 = np.concatenate([res.results[k]["t0"] for k in range(NCORES)], axis=0)
    t1 = np.concatenate([res.results[k]["t1"] for k in range(NCORES)], axis=1)
    return (t0, t1)


# revision 22
# speedup vs baseline: 2.4078x; 1.1192x over previous
"""Belief-propagation single-iteration kernel for 8 Trainium2 NeuronCores.

Problem (see reference):
    theta0: (2048, 8192) f32   clique A over (v0, v1_fine)
    theta1: (4096, 2048) f32   clique B over (v1_coarse, v2)
    idx_a, idx_b: (8192,) int64 maps fine->coarse

    marg_a = logsumexp(theta0, axis=0)                      # (8192,)
    msg_ab = segment_logsumexp(marg_a, idx_b, 4096)         # (4096,)
    marg_b = logsumexp(theta1, axis=1)                      # (4096,)
    msg_ba = marg_b[idx_a]                                  # (8192,)
    t0 = theta0 + msg_ba[None, :];  t0 -= logsumexp(t0)
    t1 = theta1 + msg_ab[:, None];  t1 -= logsumexp(t1)
    return (t0, t1)

Device algorithm works in the linear "sum-exp" domain:
    S_a[j] = sum_i exp(theta0[i,j])        (AllReduce over row shards)
    S_b[c] = sum_k exp(theta1[c,k])        (AllReduce over col shards)
    seg[c] = sum_{j: idx_b[j]=c} S_a[j]    (CSR-padded ap_gather + reduce)
    dot0 = sum_j S_a[j] * S_b[idx_a[j]]  = exp(Z0)
    dot1 = sum_c S_b[c] * seg[c]         = exp(Z1)
    t0 = theta0 + ln(S_b[idx_a[j]] / dot0)   (broadcast add over rows)
    t1 = theta1 + ln(seg[c] / dot1)          (broadcast add over cols)

Sharding: theta0 row-sharded (v0), theta1 column-sharded (v2); the small
marginal vectors are AllReduced and the message computation runs
replicated on every core.  Fully SPMD - no core-id dependence anywhere.

The S_b table is stored in a "sigma" permuted order sigma(c) =
(c % 128) * 32 + c // 128 so that the per-core [128, 32] partial-sum
tile DMAs to the collective buffer with a plain access pattern; idx_a
is remapped on the host to compensate.

Partition-broadcasts of the small tables use K=1 ones-matmuls on the PE
(a 0-stride broadcast DMA measures ~65 GB/s - far too slow).  The
msg_ba gather is split 8 ways across the GPSIMD cores (ap_gather costs
~20 ns per index per core), then reassembled through a DRAM row and
rebroadcast by matmul, with ln() applied on the PSUM chunks.
"""

import numpy as np

D0, D1F, D1C, D2 = 2048, 8192, 4096, 2048
NCORES = 8
R0 = D0 // NCORES  # theta0 rows per core (256)
C2 = D2 // NCORES  # theta1 cols per core (256)
T0T = R0 // 128  # theta0 SBUF tiles per core (2)
T1T = D1C // 128  # theta1 row tiles (32)
NCHUNK = D1F // 512  # 512-wide chunks of the fine axis (16)
PAD = 128  # zero slots appended to the S_a gather table

_cache = {}


def _build(cap, repeats=1, stage=99):
    import concourse.bacc as bacc
    import concourse.tile as tile
    from concourse import mybir

    f32 = mybir.dt.float32
    i16 = mybir.dt.int16
    Exp = mybir.ActivationFunctionType.Exp
    Ln = mybir.ActivationFunctionType.Ln
    X = mybir.AxisListType.X
    add = mybir.AluOpType.add

    nc = bacc.Bacc("TRN2", target_bir_lowering=False, debug=False, num_devices=NCORES)

    th0 = nc.dram_tensor("th0", [R0, D1F], f32, kind="ExternalInput").ap()
    th1 = nc.dram_tensor("th1", [D1C, C2], f32, kind="ExternalInput").ap()
    idxa = nc.dram_tensor("idxa", [128, D1F // 128], i16, kind="ExternalInput").ap()
    lob = nc.dram_tensor("lob", [128, 64], f32, kind="ExternalInput").ap()
    hib = nc.dram_tensor("hib", [128, 64], f32, kind="ExternalInput").ap()
    iota = nc.dram_tensor("iota", [128, 128], f32, kind="ExternalInput").ap()
    t0 = nc.dram_tensor("t0", [R0, D1F], f32, kind="ExternalOutput").ap()
    t1 = nc.dram_tensor("t1", [D1C, C2], f32, kind="ExternalOutput").ap()

    with tile.TileContext(nc) as tc:
        with (
            tc.tile_pool(name="singles", bufs=1) as singles,
            tc.tile_pool(name="big", bufs=1) as big,
            tc.tile_pool(name="stg", bufs=3) as stg,
            tc.tile_pool(name="lnp", bufs=3) as lnp,
            tc.tile_pool(name="psum", bufs=4, space="PSUM") as psum,
            tc.tile_pool(name="pss", bufs=1, space="PSUM") as pss,
            tc.tile_pool(name="dram", bufs=1, space="DRAM") as dram,
        ):
            ones = singles.tile([128, 128], f32)
            nc.vector.memset(ones, 1.0)
            ones_col = singles.tile([1, 128], f32)
            nc.vector.memset(ones_col, 1.0)
            idxa_sb = singles.tile([128, D1F // 128], i16)
            nc.sync.dma_start(out=idxa_sb, in_=idxa)
            lob_sb = singles.tile([128, 64], f32)
            nc.sync.dma_start(out=lob_sb, in_=lob)
            hib_sb = singles.tile([128, 64], f32)
            nc.sync.dma_start(out=hib_sb, in_=hib)
            iota_sb = singles.tile([128, 128], f32)
            nc.sync.dma_start(out=iota_sb, in_=iota)

            for _rep in range(repeats):
                th0_sb = big.tile([128, T0T, D1F], f32, tag="th0")
                th1_sb = big.tile([128, T1T, C2], f32, tag="th1")
                sb_tbl = big.tile([128, D1C], f32, tag="sbt")
                row_sb = singles.tile([1, D1F], f32, tag="row")
                mb_g = singles.tile([128, D1F // 8], f32, tag="mbg")
                sb_col = singles.tile([128, T1T], f32, tag="sbcol")
                sbl_col = singles.tile([128, T1T], f32, tag="sblcol")
                prod1 = singles.tile([128, T1T], f32, tag="prod1")
                sa64 = singles.tile([128, 64], f32, tag="sa64")
                mb64 = singles.tile([128, 64], f32, tag="mb64")
                d0p = singles.tile([128, 1], f32, tag="d0p")
                recip0 = singles.tile([128, 1], f32, tag="rec0")
                recip1 = singles.tile([128, 1], f32, tag="rec1")
                pr_col = singles.tile([128, 1], f32, tag="prc")
                mg_col = singles.tile([128, T1T], f32, tag="mgc")

                sa_jt = singles.tile([128, 64], f32, tag="sajt")

                ccb_in = dram.tile([1, D1C], f32, tag="ccbin")
                ccb_out = dram.tile([1, D1C], f32, tag="ccbout")
                cca_in = dram.tile([1, D1F], f32, tag="ccain")
                cca_out = dram.tile([1, D1F], f32, tag="ccaout")
                mrow_d = dram.tile([1, D1F], f32, tag="mrow")

                # ---- phase A: theta1 load -> S_b partials ----------------
                for t in range(T1T):
                    nc.sync.dma_start(out=th1_sb[:, t, :],
                                      in_=th1[128 * t:128 * (t + 1), :])
                    e1 = stg.tile([128, C2], f32, tag="t1e")
                    nc.scalar.activation(e1, th1_sb[:, t, :], Exp)
                    nc.vector.tensor_reduce(sb_col[:, t:t + 1], e1, axis=X, op=add)
                # natural [p, t] layout == sigma order sigma(c)=(c%128)*32+c//128
                nc.sync.dma_start(out=ccb_in[0, :], in_=sb_col)
                nc.gpsimd.collective_compute(
                    "AllReduce",
                    mybir.AluOpType.add,
                    replica_groups=[list(range(NCORES))],
                    ins=[ccb_in.opt()],
                    outs=[ccb_out.opt()],
                )

                if stage < 2:
                    continue
                # ---- phase B: theta0 stream in -> S_a partials -----------
                for n in range(NCHUNK):
                    sl = slice(512 * n, 512 * (n + 1))
                    for t in range(T0T):
                        nc.sync.dma_start(
                            out=th0_sb[:, t, sl],
                            in_=th0[128 * t:128 * (t + 1), sl])
                    e0 = stg.tile([128, 512], f32, tag="e0")
                    e1b = stg.tile([128, 512], f32, tag="e1b")
                    nc.scalar.activation(e0, th0_sb[:, 0, sl], Exp)
                    nc.scalar.activation(e1b, th0_sb[:, 1, sl], Exp)
                    ps = psum.tile([128, 512], f32, tag="ps")
                    nc.tensor.matmul(ps, ones, e0, start=True, stop=False)
                    nc.tensor.matmul(ps, ones, e1b, start=False, stop=True)
                    # every PSUM row holds the same column sums; ship row 0
                    sa_row = stg.tile([1, 512], f32, tag="sarow")
                    nc.vector.tensor_copy(sa_row, ps[0:1, :])
                    nc.sync.dma_start(out=cca_in[0, sl], in_=sa_row)

                if stage < 3:
                    continue
                # ---- AllReduce (S_a) -------------------------------------
                nc.gpsimd.collective_compute(
                    "AllReduce",
                    mybir.AluOpType.add,
                    replica_groups=[list(range(NCORES))],
                    ins=[cca_in.opt()],
                    outs=[cca_out.opt()],
                )

                if stage < 4:
                    continue
                # ---- S_b broadcast + msg_ba gather -----------------------
                nc.sync.dma_start(out=row_sb[0:1, :D1C], in_=ccb_out[0:1, :])
                for n in range(D1C // 512):
                    sl = slice(512 * n, 512 * (n + 1))
                    ps = psum.tile([128, 512], f32, tag="ps")
                    nc.tensor.matmul(ps, ones_col, row_sb[0:1, sl],
                                     start=True, stop=True)
                    if n % 2 == 0:
                        nc.vector.tensor_copy(sb_tbl[:, sl], ps)
                    else:
                        nc.scalar.copy(sb_tbl[:, sl], ps)
                if stage >= 5:
                    # msg_ba gather, split 8 ways: core k's partition group
                    # gathers S_b[sigma(idx_a[j])] for j in [1024k, 1024k+1024)
                    nc.gpsimd.ap_gather(
                        out_ap=mb_g[:].rearrange("p (n d) -> p n d", d=1),
                        in_ap=sb_tbl[:].rearrange("p (n d) -> p n d", d=1),
                        idxs_ap=idxa_sb[:],
                        channels=128, num_elems=D1C, d=1, num_idxs=D1F // 8)
                    for k in range(8):
                        nc.sync.dma_start(
                            out=mrow_d[0, 1024 * k:1024 * (k + 1)],
                            in_=mb_g[16 * k:16 * k + 1, :])
                    nc.sync.dma_start(
                        out=mb64,
                        in_=mrow_d[0, :].rearrange("(p t) -> p t", t=64))
                nc.sync.dma_start(
                    out=sbl_col,
                    in_=ccb_out[0, :].rearrange("(p t) -> p t", t=T1T))
                # small c-layout vectors for the normalizer dots
                nc.sync.dma_start(
                    out=sa64,
                    in_=cca_out[0, :].rearrange("(p t) -> p t", t=64))
                # S_a in (j % 128, j // 128) layout for the one-hot seg sum
                nc.sync.dma_start(
                    out=sa_jt,
                    in_=cca_out[0, :].rearrange("(t p) -> p t", p=128))
                if stage >= 5:
                    # segment sums on the PE: psum_seg[p, t] = seg[128t + p]
                    #   = sum_j 1[idx_b[j] % 128 = p] * 1[idx_b[j]//128 = t]
                    #         * S_a[j]
                    psum_seg = pss.tile([128, T1T], f32, tag="segps")
                    for jt in range(64):
                        loh = stg.tile([128, 128], f32, tag="loh")
                        nc.vector.tensor_scalar(
                            loh, iota_sb, lob_sb[:, jt:jt + 1], None,
                            op0=mybir.AluOpType.is_equal)
                        him = stg.tile([128, T1T], f32, tag="him")
                        nc.vector.tensor_scalar(
                            him, iota_sb[:, :T1T], hib_sb[:, jt:jt + 1],
                            sa_jt[:, jt:jt + 1],
                            op0=mybir.AluOpType.is_equal,
                            op1=mybir.AluOpType.mult)
                        nc.tensor.matmul(psum_seg, loh, him,
                                         start=(jt == 0), stop=(jt == 63))

                if stage < 6:
                    continue
                # ---- t0 path: dot0, rebroadcast msg_ba row, ln, add ------
                nc.vector.tensor_mul(mb64, mb64, sa64)
                nc.vector.tensor_reduce(d0p, mb64, axis=X, op=add)
                d0ps = pss.tile([128, 1], f32, tag="d0ps")
                nc.tensor.matmul(d0ps, ones, d0p, start=True, stop=True)
                nc.vector.reciprocal(out=recip0, in_=d0ps)

                nc.sync.dma_start(out=row_sb[0:1, :], in_=mrow_d[0:1, :])
                for n in range(NCHUNK):
                    sl = slice(512 * n, 512 * (n + 1))
                    ps = psum.tile([128, 512], f32, tag="ps")
                    nc.tensor.matmul(ps, ones_col, row_sb[0:1, sl],
                                     start=True, stop=True)
                    lnc = lnp.tile([128, 512], f32, tag="lnc")
                    # msgba_adj = ln(S_b[idx_a[j]] / dot0), from PSUM
                    nc.scalar.activation(lnc, ps, Ln, scale=recip0[:, 0:1])
                    for t in range(T0T):
                        nc.vector.tensor_add(th0_sb[:, t, sl],
                                             th0_sb[:, t, sl], lnc)
                        nc.sync.dma_start(out=t0[128 * t:128 * (t + 1), sl],
                                          in_=th0_sb[:, t, sl])

                if stage < 7:
                    continue
                # ---- t1 path: dot1, ln, add ------------------------------
                nc.vector.tensor_mul(prod1, psum_seg, sbl_col)
                nc.vector.tensor_reduce(pr_col, prod1, axis=X, op=add)
                prps = pss.tile([128, 1], f32, tag="prps")
                nc.tensor.matmul(prps, ones, pr_col, start=True, stop=True)
                nc.vector.reciprocal(out=recip1, in_=prps)
                # msgab_col[p, t] = ln(seg[128t+p] / dot1)
                nc.scalar.activation(mg_col, psum_seg, Ln, scale=recip1[:, 0:1])
                for t in range(T1T):
                    nc.vector.tensor_scalar_add(th1_sb[:, t, :], th1_sb[:, t, :],
                                                mg_col[:, t:t + 1])
                    nc.sync.dma_start(out=t1[128 * t:128 * (t + 1), :],
                                      in_=th1_sb[:, t, :])

    nc.compile()
    return nc


def _prep_inputs(theta0, theta1, idx_a, idx_b):
    """Host-side sharding + index-table construction (no float math)."""
    theta0 = np.ascontiguousarray(np.asarray(theta0, dtype=np.float32))
    theta1 = np.ascontiguousarray(np.asarray(theta1, dtype=np.float32))
    ia = np.asarray(idx_a).astype(np.int64)
    ib = np.asarray(idx_b).astype(np.int64)

    # sigma-permuted idx_a (S_b table is stored in sigma order), split 8 ways:
    # core k's group holds j in [1024k, 1024k+1024) wrapped over 16 partitions
    sig_ia = ((ia % 128) * 32 + ia // 128).astype(np.int16)
    idxa_w = np.ascontiguousarray(
        sig_ia.reshape(8, 64, 16).transpose(0, 2, 1).reshape(128, 64))

    # idx_b lo/hi tables in (j % 128, j // 128) layout for the one-hot
    # segment sum: seg lands as [p = c % 128, t = c // 128]
    ibm = ib.reshape(64, 128).T
    lob_w = np.ascontiguousarray((ibm % 128).astype(np.float32))
    hib_w = np.ascontiguousarray((ibm // 128).astype(np.float32))
    iota_w = np.tile(np.arange(128, dtype=np.float32)[None, :], (128, 1))

    in_maps = []
    for k in range(NCORES):
        in_maps.append({
            "th0": theta0[R0 * k:R0 * (k + 1), :],
            "th1": np.ascontiguousarray(theta1[:, C2 * k:C2 * (k + 1)]),
            "idxa": idxa_w,
            "lob": lob_w,
            "hib": hib_w,
            "iota": iota_w,
        })
    return 0, in_maps


def get_program(cap, repeats=1):
    key = ("prog", cap, repeats)
    if key not in _cache:
        _cache[key] = _build(cap, repeats)
    return _cache[key]


def kernel(theta0, theta1, idx_a, idx_b):
    from concourse.bass_utils import run_bass_kernel_spmd

    cap, in_maps = _prep_inputs(theta0, theta1, idx_a, idx_b)
    nc = get_program(cap)
    res = run_bass_kernel_spmd(nc, in_maps, list(range(NCORES)))
    t0 = np.concatenate([res.results[k]["t0"] for k in range(NCORES)], axis=0)
    t1 = np.concatenate([res.results[k]["t1"] for k in range(NCORES)], axis=1)
    return (t0, t1)
